# revision 17
# baseline (speedup 1.0000x reference)
"""Self-contained Trainium2 kernel for nn_Attention_24799141167815.

Cosine-similarity attention (Swin-v2 style) with continuous position bias.
Data-parallel over batch B=8 across 8 NeuronCores (core b handles batch b).

Design notes (v2):
  - rel-bias is RAW (not exp'd), log2e-scaled, fp8e4, and added into the
    scores PSUM via an identity-stationary matmul for IDENT_KTS; for
    DVE_KTS/GP_KTS the exp'd bias (bf16) is multiplied into eT on the
    vector / gpsimd engine instead, balancing the three engines.
  - score matmuls for the two q blocks are row-packed (qb0 on partitions
    0-63, qb1 on 64-127, issued back-to-back so both 64-deep matmuls
    co-issue in the PE array).
  - norms: per head-pair ONE psum tile holds q-sumsq at partition offset
    0 and k-sumsq at offset 32 (32-wide zero-padded stationaries), so a
    single wide ACT Sqrt covers both.  All four head-pairs' norm chains
    run before the first Exp, so the ACT table is loaded exactly twice
    (Sqrt once, Exp once) for the whole kernel.
  - the per-token norm reciprocals are bounced through a tiny DRAM
    scratch and broadcast back to 64/128 partitions with stride-0 DRAM
    DMA reads -- no PE broadcast matmuls at all.
  - reciprocals via reciprocal_approx_fast, always from SBUF (custom
    DVE ops misread PSUM at partition offsets).
  - division is batched per head: one [1,1024] reciprocal + one gpsimd
    partition broadcast covers both q blocks.
"""

import os
import numpy as np
import ml_dtypes

import concourse.bass as bass
import concourse.mybir as mybir
import concourse.tile as tile
from concourse import bacc
from concourse.bass_utils import run_bass_kernel_spmd

F32 = mybir.dt.float32
BF16 = mybir.dt.bfloat16
FP16 = mybir.dt.float16
FP8 = mybir.dt.float8e4
AF = mybir.ActivationFunctionType
ALU = mybir.AluOpType

B, N, C = 8, 1024, 512
H, HD = 8, 64
NT = N // 128     # 8 key tiles
CB = C // 128     # 4 cin blocks
QB = 2            # q blocks of 512
IDENT_KTS = (0, 1, 3, 4, 6, 7)  # bias via fp8 identity matmul into PSUM
DVE_KTS = (2, 5)              # bias via DVE multiply of exp'd bias
GP_KTS = ()                   # (gpsimd multiply causes library-reload thrash
                              #  with PartitionBroadcast -- keep gpsimd
                              #  broadcast-only)
EXPB_KTS = tuple(sorted(DVE_KTS + GP_KTS))
EXPB_IDX = {kt: i for i, kt in enumerate(EXPB_KTS)}
NB_BF16 = np.dtype(ml_dtypes.bfloat16)
NB_FP16 = np.dtype(np.float16)
NB_FP8 = np.dtype(ml_dtypes.float8_e4m3)
LOG2E = float(np.log2(np.e))
LN2 = float(np.log(2.0))

_CACHE = {}


def _build(reps=1):
    nc = bacc.Bacc("TRN2", target_bir_lowering=False)

    xT_d = nc.declare_dram_parameter("xT", [C, N], FP16, isOutput=False)
    wqkT_d = nc.declare_dram_parameter("wqkT", [C, 2 * C], FP16, isOutput=False)
    wvT_d = nc.declare_dram_parameter("wvT", [C, C], FP16, isOutput=False)
    qkb_d = nc.declare_dram_parameter("qkb", [2 * C, 1], F32, isOutput=False)
    vbrow_d = nc.declare_dram_parameter("vbrow", [1, C], F32, isOutput=False)
    qev_d = nc.declare_dram_parameter("qev", [64, H], FP16, isOutput=False)
    sclq_d = nc.declare_dram_parameter("sclq", [128, CB], F32, isOutput=False)
    projwT_d = nc.declare_dram_parameter("projwT", [C, C], FP16, isOutput=False)
    projbrow_d = nc.declare_dram_parameter("projbrow", [1, C], F32, isOutput=False)
    biasT_d = nc.declare_dram_parameter(
        "biasT", [H, len(IDENT_KTS) * 128, N], FP8, isOutput=False)
    expbT_d = nc.declare_dram_parameter(
        "expbT", [H, len(EXPB_KTS) * 128, N], BF16, isOutput=False)
    bsum32_d = nc.declare_dram_parameter("bsum32", [128, 32], FP16, isOutput=False)
    identf8_d = nc.declare_dram_parameter("identf8", [128, 128], FP8, isOutput=False)
    rrd_d = nc.declare_dram_parameter("rrd", [4 * CB, N], FP16, isOutput=True)
    out_d = nc.declare_dram_parameter("out", [N, C], F32, isOutput=True)

    with tile.TileContext(nc) as tc:
        with (
            tc.tile_pool(name="persist", bufs=1) as persist,
            tc.tile_pool(name="sqp", bufs=2) as sqp,
            tc.tile_pool(name="psbp", bufs=2) as psbp,
            tc.tile_pool(name="ebias", bufs=2) as ebias,
            tc.tile_pool(name="expt", bufs=2) as expt_pool,
            tc.tile_pool(name="small", bufs=1) as small,
            tc.tile_pool(name="osbp", bufs=2) as osbp,
            tc.tile_pool(name="ps_big", bufs=3, space="PSUM") as ps_big,
            tc.tile_pool(name="ps_av", bufs=2, space="PSUM") as ps_av,
        ):
            # ---------------- load constants / weights ----------------
            # big-chunk DMAs (>=1KB contiguous per partition line) split
            # across the two hardware queues: sync gets x cols 0:512 +
            # weights, scalar gets x cols 512:1024 (+ biases later)
            xT = persist.tile([128, CB, N], FP16, tag="xT")
            nc.sync.dma_start(
                out=xT[:, :, 0:512],
                in_=xT_d.rearrange("(cb p) n -> p cb n", p=128)[:, :, 0:512])
            nc.scalar.dma_start(
                out=xT[:, :, 512:1024],
                in_=xT_d.rearrange("(cb p) n -> p cb n", p=128)[:, :, 512:1024])
            wqkT = persist.tile([128, CB, 2 * C], FP16, tag="wqkT")
            nc.sync.dma_start(
                out=wqkT, in_=wqkT_d.rearrange("(cb p) f -> p cb f", p=128))
            wvT = persist.tile([128, CB, C], FP16, tag="wvT")
            nc.sync.dma_start(
                out=wvT, in_=wvT_d.rearrange("(cb p) f -> p cb f", p=128))
            qkb = persist.tile([128, 2 * CB], F32, tag="qkb")
            nc.sync.dma_start(
                out=qkb, in_=qkb_d.rearrange("(fb p) one -> p (fb one)", p=128))
            sclq = persist.tile([128, CB], F32, tag="sclq")
            nc.sync.dma_start(out=sclq, in_=sclq_d[:])
            bsum32 = persist.tile([128, 32], FP16, tag="bsum32")
            nc.sync.dma_start(out=bsum32, in_=bsum32_d[:])
            identf8 = persist.tile([128, 128], FP8, tag="identf8")
            nc.sync.dma_start(out=identf8, in_=identf8_d[:])
            vb_bc = persist.tile([128, C], F32, tag="vb_bc")
            nc.sync.dma_start(out=vb_bc, in_=vbrow_d[:].to_broadcast((128, C)))
            qev = persist.tile([64, H], FP16, tag="qev")
            nc.sync.dma_start(out=qev, in_=qev_d[:])

            for rep in range(reps):
                qkT = persist.tile([128, 2 * CB, N], FP16, tag="qkT")
                qsTd = persist.tile([128, H, N], FP16, tag="qsTd")
                knTd = persist.tile([128, H, N], FP16, tag="knTd")
                v_sb = persist.tile([128, NT, H, HD + 1], BF16, tag="v_sb")
                qekn = persist.tile([128, H, NT], F32, tag="qekn")
                outhT = persist.tile([128, CB, N], FP16, tag="outhT")

                def qkv(hp):
                    # q (fb=hp) and k (fb=CB+hp) projections, DVE eviction
                    for half in range(2):
                        fb = half * CB + hp
                        for qb in range(QB):
                            ps = ps_big.tile(
                                [128, 1024], F32, tag="ps_big",
                                name=f"psqkv{hp}{half}{qb}")
                            for cb in range(CB):
                                nc.tensor.matmul(
                                    ps[:, 0:512],
                                    wqkT[:, cb, fb * 128:(fb + 1) * 128],
                                    xT[:, cb, qb * 512:(qb + 1) * 512],
                                    start=(cb == 0), stop=(cb == CB - 1),
                                )
                            nc.vector.tensor_scalar(
                                out=qkT[:, fb, qb * 512:(qb + 1) * 512],
                                in0=ps[:, 0:512], scalar1=qkb[:, fb:fb + 1],
                                scalar2=None, op0=ALU.add)

                def vproj():
                    nc.vector.memset(v_sb[:, :, :, HD:HD + 1], 1.0)
                    for tb in range(NT):
                        ps = ps_big.tile([128, 1024], F32, tag="ps_big",
                                         name=f"psv{tb}")
                        for cb in range(CB):
                            nc.tensor.matmul(
                                ps[:, 0:512],
                                xT[:, cb, tb * 128:(tb + 1) * 128],
                                wvT[:, cb, :],
                                start=(cb == 0), stop=(cb == CB - 1),
                            )
                        nc.vector.tensor_add(
                            v_sb[:, tb, :, 0:HD],
                            ps[:, 0:512].rearrange("p (h d) -> p h d", h=H),
                            vb_bc.rearrange("p (h d) -> p h d", h=H),
                        )

                def norms_mm(hp):
                    # squared sums for q and k halves land in ONE psum tile
                    # (q heads at partitions 0-1, k heads at 32-33, rest
                    # zeroed by the 32-wide zero-padded stationary), then a
                    # single wide Sqrt covers both.
                    sqs = []
                    for half in range(2):
                        fb = half * CB + hp
                        sq = sqp.tile([128, N], FP16, tag="sq",
                                      name=f"sq{hp}{half}")
                        nc.vector.tensor_mul(sq, qkT[:, fb, :], qkT[:, fb, :])
                        sqs.append(sq)
                    nps = ps_big.tile([128, 1024], F32, tag="ps_big",
                                      name=f"nps{hp}")
                    for half in range(2):
                        for qb in range(QB):
                            nc.tensor.matmul(
                                nps[32 * half:32 * half + 32,
                                    qb * 512:(qb + 1) * 512],
                                bsum32, sqs[half][:, qb * 512:(qb + 1) * 512],
                                start=True, stop=True)
                    srt = sqp.tile([64, N], F32, tag="srt", bufs=4,
                                   name=f"srt{hp}")
                    nc.scalar.activation(
                        out=srt, in_=nps[0:64, :],
                        func=AF.Sqrt, bias=0.0, scale=1.0)
                    return srt

                def norms_chain(hp, srt):
                    # custom DVE ops misread at partition offsets: run ONE
                    # base-0 reciprocal over rows 0-33 (rows 2-31 are zeros
                    # whose inf recips get clamped and never used)
                    rr = sqp.tile([34, N], F32, tag="rr", bufs=1,
                                  name=f"rr{hp}")
                    nc.vector.reciprocal_approx_fast(rr, srt[0:34, :])
                    rr16 = sqp.tile([34, N], FP16, tag="rr16",
                                    name=f"rr16{hp}")
                    nc.vector.tensor_scalar(
                        out=rr16, in0=rr, scalar1=1e12, scalar2=None,
                        op0=ALU.min)
                    nc.gpsimd.dma_start(
                        out=rrd_d[hp * 4:hp * 4 + 2, :], in_=rr16[0:2, :])
                    nc.gpsimd.dma_start(
                        out=rrd_d[hp * 4 + 2:hp * 4 + 4, :],
                        in_=rr16[32:34, :])

                def scaleqk2(hp):
                    # broadcast the norm recips back from DRAM scratch with
                    # stride-0 reads (rides the same gpsimd hwdge queue as
                    # the writes in norms2, so ordering is FIFO-safe), then
                    # scale both heads of the pair in one [128,1024] op.
                    for half, dst in ((0, qsTd), (1, knTd)):
                        fb = half * CB + hp
                        r0 = hp * 4 + half * 2
                        psb = psbp.tile([128, N], FP16, tag="psb",
                                        name=f"psb{hp}{half}")
                        nc.gpsimd.dma_start(
                            out=psb[0:64, :],
                            in_=rrd_d[r0:r0 + 1, :].to_broadcast((64, N)))
                        nc.gpsimd.dma_start(
                            out=psb[64:128, :],
                            in_=rrd_d[r0 + 1:r0 + 2, :].to_broadcast((64, N)))
                        packed = psbp.tile([128, N], FP16, tag="packed",
                                           name=f"packed{hp}{half}")
                        if half == 0:
                            nc.vector.scalar_tensor_tensor(
                                out=packed, in0=qkT[:, fb, :],
                                scalar=sclq[:, hp:hp + 1], in1=psb,
                                op0=ALU.mult, op1=ALU.mult)
                        else:
                            nc.vector.tensor_mul(packed, qkT[:, fb, :], psb)
                        # duplicate each head's 64 rows into both row halves
                        # so score matmuls can row-pack the two q blocks
                        # (spread across both hw queues)
                        for s in range(2):
                            h = 2 * hp + s
                            nc.sync.dma_start(
                                out=dst[:, h, :][0:64],
                                in_=packed[s * 64:(s + 1) * 64, :])
                            nc.scalar.dma_start(
                                out=dst[:, h, :][64:128],
                                in_=packed[s * 64:(s + 1) * 64, :])

                def qekn_calc(hp):
                    for s in range(2):
                        h = 2 * hp + s
                        psq = ps_big.tile([128, 1024], F32, tag="ps_big",
                                          name=f"psq{h}")
                        for kt in range(NT):
                            nc.tensor.matmul(
                                psq[:, 2 * kt:2 * kt + 1],
                                knTd[:, h, kt * 128:(kt + 1) * 128][0:64],
                                qev[:, h:h + 1],
                                start=True, stop=True)
                        nc.vector.tensor_copy(
                            qekn[:, h, :],
                            psq[:, 0:2 * NT].rearrange(
                                "p (a two) -> p a two", two=2)[:, :, 0])

                bias_tiles = {}

                def prefetch_bias(h):
                    # bias DMAs ride the scalar (ACT) hwdge queue so the big
                    # fp8 streams never block weight loads on the sync queue
                    bt = ebias.tile([128, len(IDENT_KTS), N], FP8, tag="bias",
                                    name=f"bias{h}")
                    nc.scalar.dma_start(
                        out=bt,
                        in_=biasT_d[h].rearrange("(kt p) q -> p kt q", p=128))
                    ebt = ebias.tile([128, len(EXPB_KTS), N], BF16,
                                     tag="expb", name=f"expb{h}")
                    nc.scalar.dma_start(
                        out=ebt,
                        in_=expbT_d[h].rearrange("(kt p) q -> p kt q", p=128))
                    bias_tiles[h] = (bt, ebt)

                pav_tiles = {}

                def group_main(h):
                    # attention for one head, both q blocks at once
                    bias_t, expb_t = bias_tiles.pop(h)
                    eT = expt_pool.tile([128, NT, N], BF16, tag="eT",
                                        name=f"eT{h}")
                    pavs = [ps_av.tile([HD + 1, 512], F32, tag="ps_av",
                                       name=f"pav{h}{i}") for i in range(2)]
                    pav_tiles[h] = pavs

                    def scores(kt):
                        ident = kt in IDENT_KTS
                        ps = ps_big.tile([128, 1024], F32, tag="ps_big",
                                         name=f"pssc{h}{kt}")
                        if ident:
                            j = IDENT_KTS.index(kt)
                            for qb in range(QB):
                                nc.tensor.matmul(
                                    ps[:, qb * 512:(qb + 1) * 512], identf8,
                                    bias_t[:, j, qb * 512:(qb + 1) * 512],
                                    start=True, stop=False)
                        # row-packed pair: qb0 on rows 0-63, qb1 on 64-127
                        for qb in range(QB):
                            ro = qb * 64
                            nc.tensor.matmul(
                                ps[:, qb * 512:(qb + 1) * 512],
                                knTd[:, h, kt * 128:(kt + 1) * 128][ro:ro + 64],
                                qsTd[:, h,
                                     qb * 512:(qb + 1) * 512][ro:ro + 64],
                                start=not ident, stop=True,
                            )
                        nc.scalar.activation(
                            out=eT[:, kt, :], in_=ps, func=AF.Exp,
                            bias=qekn[:, h, kt:kt + 1], scale=LN2)
                        if kt in DVE_KTS:
                            nc.vector.tensor_mul(
                                eT[:, kt, :], eT[:, kt, :],
                                expb_t[:, EXPB_IDX[kt], :])
                        elif kt in GP_KTS:
                            nc.gpsimd.tensor_mul(
                                eT[:, kt, :], eT[:, kt, :],
                                expb_t[:, EXPB_IDX[kt], :])

                    def av(kt):
                        for qb in range(QB):
                            nc.tensor.matmul(
                                pavs[qb],
                                v_sb[:, kt, h, :],
                                eT[:, kt, qb * 512:(qb + 1) * 512],
                                start=(kt == 0), stop=(kt == NT - 1),
                            )

                    # AV lags scores by 4 kt so the previous group's division
                    # (which holds the pav slots) has time to finish
                    for kt in range(NT):
                        scores(kt)
                        if kt >= 4:
                            av(kt - 4)
                    for kt in range(NT - 4, NT):
                        av(kt)

                def group_div(h):
                    # division for one head, both q blocks batched:
                    # one reciprocal + one partition broadcast
                    hp, sub = h // 2, h % 2
                    po = sub * 64
                    pav0, pav1 = pav_tiles[h]
                    den = small.tile([1, 1024], F32, tag="dens",
                                     name=f"den{h}")
                    nc.vector.tensor_copy(den[:, 0:512], pav0[HD:HD + 1, :])
                    nc.vector.tensor_copy(den[:, 512:1024], pav1[HD:HD + 1, :])
                    rrec = small.tile([1, 1024], F32, tag="rrec",
                                      name=f"rrec{h}")
                    nc.vector.reciprocal_approx_fast(rrec, den)
                    rrb = small.tile([HD, 1024], F32, tag="rrb",
                                     name=f"rrb{h}")
                    nc.gpsimd.partition_broadcast(rrb, rrec)
                    for qb, pav in ((0, pav0), (1, pav1)):
                        nc.vector.scalar_tensor_tensor(
                            out=outhT[:, hp,
                                      qb * 512:(qb + 1) * 512][po:po + 64],
                            in0=pav[0:HD, :], scalar=1.0,
                            in1=rrb[:, qb * 512:(qb + 1) * 512],
                            op0=ALU.mult, op1=ALU.mult)

                def group_div_qb(h, qb):
                    # unbatched variant for the final head so proj can start
                    # after the first q block's division
                    hp, sub = h // 2, h % 2
                    po = sub * 64
                    pav = pav_tiles[h][qb]
                    den = small.tile([1, 512], F32, tag="dens1",
                                     name=f"den1{h}{qb}")
                    nc.vector.tensor_copy(den, pav[HD:HD + 1, :])
                    rrec = small.tile([1, 512], F32, tag="rrec1",
                                      name=f"rrec1{h}{qb}")
                    nc.vector.reciprocal_approx_fast(rrec, den)
                    rrb = small.tile([HD, 512], F32, tag="rrb1",
                                     name=f"rrb1{h}{qb}")
                    nc.gpsimd.partition_broadcast(rrb, rrec)
                    nc.vector.scalar_tensor_tensor(
                        out=outhT[:, hp, qb * 512:(qb + 1) * 512][po:po + 64],
                        in0=pav[0:HD, :], scalar=1.0, in1=rrb,
                        op0=ALU.mult, op1=ALU.mult)

                def proj(tb):
                    ps = ps_big.tile([128, 1024], F32, tag="ps_big",
                                     name=f"pso{tb}")
                    for fb in range(CB):
                        nc.tensor.matmul(
                            ps[:, 0:512],
                            outhT[:, fb, tb * 128:(tb + 1) * 128],
                            projwT[:, fb, :],
                            start=(fb == 0), stop=(fb == CB - 1),
                        )
                    osb = osbp.tile([128, C], F32, tag="osb", name=f"osb{tb}")
                    nc.vector.tensor_add(osb, ps[:, 0:512], projb_bc)
                    nc.sync.dma_start(
                        out=out_d[tb * 128:(tb + 1) * 128, :], in_=osb)

                # prologue: all four head-pairs' QKV + norm chains run before
                # any attention so the ACT queue is [sqrt x4, exp x64] with
                # exactly two table loads.  All QKV psum evictions and sq
                # squares are emitted BEFORE any norm-chain DVE op so the
                # chains never head-of-line-block the PE's psum recycling.
                # bias/proj weight DMAs are emitted late so the scheduler
                # can't run them ahead of the critical x/w loads.
                qkv(0)
                qkv(1)
                srt0 = norms_mm(0)
                qkv(2)
                srt1 = norms_mm(1)
                qkv(3)
                srt2 = norms_mm(2)
                srt3 = norms_mm(3)
                norms_chain(0, srt0)
                scaleqk2(0)
                norms_chain(1, srt1)
                scaleqk2(1)
                prefetch_bias(0)
                prefetch_bias(1)
                norms_chain(2, srt2)
                scaleqk2(2)
                norms_chain(3, srt3)
                scaleqk2(3)
                vproj()
                qekn_calc(0)
                qekn_calc(1)
                qekn_calc(2)
                qekn_calc(3)
                projwT = persist.tile([128, CB, C], FP16, tag="projwT")
                nc.scalar.dma_start(
                    out=projwT,
                    in_=projwT_d.rearrange("(cb p) f -> p cb f", p=128))
                projb_bc = persist.tile([128, C], F32, tag="projb_bc")
                nc.scalar.dma_start(
                    out=projb_bc, in_=projbrow_d[:].to_broadcast((128, C)))
                for h in range(H):
                    if h + 2 < H:
                        prefetch_bias(h + 2)
                    # division for the previous head is emitted BEFORE this
                    # head's group so its gpsimd broadcast isn't queued
                    # behind this head's gpsimd eT multiply
                    if h >= 1:
                        group_div(h - 1)
                    group_main(h)
                group_div_qb(H - 1, 0)
                for tb in range(NT // 2):
                    proj(tb)
                group_div_qb(H - 1, 1)
                for tb in range(NT // 2, NT):
                    proj(tb)

    nc.compile()
    return nc


def _host_prep(inputs):
    """Host-side layout/scalar prep. Returns per-core input maps."""
    x = np.asarray(inputs["x"], dtype=np.float32)
    qkv_w = np.asarray(inputs["qkv_w"], dtype=np.float32)
    qkv_b = np.asarray(inputs["qkv_b"], dtype=np.float32)
    proj_w = np.asarray(inputs["proj_w"], dtype=np.float32)
    proj_b = np.asarray(inputs["proj_b"], dtype=np.float32)
    temp = np.asarray(inputs["temperature"], dtype=np.float32).reshape(H)
    qe = np.asarray(inputs["query_embedding"], dtype=np.float32).reshape(H, HD)
    tab = np.asarray(inputs["relative_coords_table"], dtype=np.float32)
    idx = np.asarray(inputs["relative_pos_index"])
    f1w = np.asarray(inputs["cpb_fc1_w"], dtype=np.float32)
    f1b = np.asarray(inputs["cpb_fc1_b"], dtype=np.float32)
    f2w = np.asarray(inputs["cpb_fc2_w"], dtype=np.float32)
    f2b = np.asarray(inputs["cpb_fc2_b"], dtype=np.float32)
    sls = np.asarray(inputs["seq_length_scale"], dtype=np.float32)

    scale = (np.logaddexp(0.0, temp) * sls[0]).astype(np.float32)

    hidden = np.maximum(tab @ f1w.T + f1b, 0.0)
    bias_tab = ((hidden @ f2w.T + f2b) * LOG2E).astype(np.float32)  # (T, H)
    bias = bias_tab[idx]                                            # (q, k, H)
    biasT = np.ascontiguousarray(np.transpose(bias, (2, 1, 0)))     # (H, k, q)
    b4 = biasT.reshape(H, NT, 128, N)
    expbT = np.exp(b4[:, EXPB_KTS].astype(np.float32) * LN2).astype(
        NB_BF16).reshape(H, len(EXPB_KTS) * 128, N)
    biasT = np.ascontiguousarray(b4[:, IDENT_KTS]).astype(NB_FP8).reshape(
        H, len(IDENT_KTS) * 128, N)

    wqkT = np.ascontiguousarray(qkv_w[:2 * C].T).astype(NB_FP16)   # (cin, 1024)
    wvT = np.ascontiguousarray(qkv_w[2 * C:].T).astype(NB_FP16)    # (cin, 512)
    projwT = np.ascontiguousarray(proj_w.T).astype(NB_FP16)        # (cin, 512)
    qkb = qkv_b[:2 * C].reshape(2 * C, 1).copy()
    vbrow = qkv_b[2 * C:].reshape(1, C).copy()
    projbrow = proj_b.reshape(1, C).copy()
    # qev[d, h] = qe[h, d]*scale[h]; knTd[0:64].T @ qev[:, h] gives the
    # per-key k_hat . qe*scale offset.
    qev = np.ascontiguousarray((qe * scale[:, None]).T).astype(NB_FP16)
    # sclq[p, hp] = scale[2*hp + (p>=64)] * log2e, applied per feature row
    # during the packed q-scale STT
    sclq = np.zeros((128, CB), dtype=np.float32)
    for hp in range(CB):
        sclq[0:64, hp] = scale[2 * hp] * LOG2E
        sclq[64:128, hp] = scale[2 * hp + 1] * LOG2E

    bsum32 = np.zeros((128, 32), dtype=NB_FP16)
    bsum32[:64, 0] = 1.0
    bsum32[64:, 1] = 1.0
    identf8 = np.eye(128, dtype=NB_FP8)

    shared = dict(
        wqkT=wqkT, wvT=wvT, qkb=qkb, vbrow=vbrow, qev=qev,
        sclq=sclq, projwT=projwT, projbrow=projbrow, biasT=biasT,
        expbT=expbT, bsum32=bsum32, identf8=identf8,
    )
    in_maps = []
    for b in range(B):
        m = dict(shared)
        m["xT"] = np.ascontiguousarray(x[b].T).astype(NB_FP16)
        in_maps.append(m)
    return in_maps


def get_nc(reps=1):
    key = ("nc", reps)
    if key not in _CACHE:
        _CACHE[key] = _build(reps)
    return _CACHE[key]


def kernel(**inputs) -> np.ndarray:
    nc = get_nc()
    in_maps = _host_prep(inputs)
    res = run_bass_kernel_spmd(nc, in_maps, core_ids=list(range(B)))
    out = np.stack([res.results[b]["out"] for b in range(B)], axis=0)
    return out.astype(np.float32)


# revision 33
# speedup vs baseline: 1.2541x; 1.2541x over previous
"""Self-contained Trainium2 kernel for nn_Attention_24799141167815.

Cosine-similarity attention (Swin-v2 style) with continuous position bias.
Data-parallel over batch B=8 across 8 NeuronCores (core b handles batch b).

Design notes (v2):
  - rel-bias is RAW (not exp'd), log2e-scaled, fp8e4, and added into the
    scores PSUM via an identity-stationary matmul for IDENT_KTS; for
    DVE_KTS/GP_KTS the exp'd bias (bf16) is multiplied into eT on the
    vector / gpsimd engine instead, balancing the three engines.
  - score matmuls for the two q blocks are row-packed (qb0 on partitions
    0-63, qb1 on 64-127, issued back-to-back so both 64-deep matmuls
    co-issue in the PE array).
  - norms: per head-pair ONE psum tile holds q-sumsq at partition offset
    0 and k-sumsq at offset 32 (32-wide zero-padded stationaries), so a
    single wide ACT Sqrt covers both.  All four head-pairs' norm chains
    run before the first Exp, so the ACT table is loaded exactly twice
    (Sqrt once, Exp once) for the whole kernel.
  - the per-token norm reciprocals are bounced through a tiny DRAM
    scratch and broadcast back to 64/128 partitions with stride-0 DRAM
    DMA reads -- no PE broadcast matmuls at all.
  - reciprocals via reciprocal_approx_fast, always from SBUF (custom
    DVE ops misread PSUM at partition offsets).
  - division is batched per head: one [1,1024] reciprocal + one gpsimd
    partition broadcast covers both q blocks.
"""

import os
import numpy as np
import ml_dtypes

import concourse.bass as bass
import concourse.mybir as mybir
import concourse.tile as tile
from concourse import bacc
from concourse.bass_utils import run_bass_kernel_spmd

F32 = mybir.dt.float32
BF16 = mybir.dt.bfloat16
FP16 = mybir.dt.float16
FP8 = mybir.dt.float8e4
AF = mybir.ActivationFunctionType
ALU = mybir.AluOpType

B, N, C = 8, 1024, 512
H, HD = 8, 64
NT = N // 128     # 8 key tiles
CB = C // 128     # 4 cin blocks
QB = 2            # q blocks of 512
IDENT_KTS = (0, 1, 3, 4, 6, 7)  # bias via fp8 identity matmul into PSUM
DVE_KTS = (2, 5)              # bias via DVE multiply of exp'd bias
GP_KTS = ()                   # (gpsimd multiply causes library-reload thrash
                              #  with PartitionBroadcast -- keep gpsimd
                              #  broadcast-only)
EXPB_KTS = tuple(sorted(DVE_KTS + GP_KTS))
EXPB_IDX = {kt: i for i, kt in enumerate(EXPB_KTS)}
NB_BF16 = np.dtype(ml_dtypes.bfloat16)
NB_FP16 = np.dtype(np.float16)
NB_FP8 = np.dtype(ml_dtypes.float8_e4m3)
LOG2E = float(np.log2(np.e))
LN2 = float(np.log(2.0))

_CACHE = {}


def _build(reps=1):
    nc = bacc.Bacc("TRN2", target_bir_lowering=False)

    xT_d = nc.declare_dram_parameter("xT", [C, N], FP16, isOutput=False)
    wqkT_d = nc.declare_dram_parameter("wqkT", [C, 2 * C], FP16, isOutput=False)
    wvT_d = nc.declare_dram_parameter("wvT", [C, C], FP16, isOutput=False)
    qkb_d = nc.declare_dram_parameter("qkb", [2 * C, 1], F32, isOutput=False)
    vbrow_d = nc.declare_dram_parameter("vbrow", [1, 2 * C], F32, isOutput=False)
    qev_d = nc.declare_dram_parameter("qev", [128, H], FP16, isOutput=False)
    sclq_d = nc.declare_dram_parameter("sclq", [128, CB], F32, isOutput=False)
    projwT_d = nc.declare_dram_parameter("projwT", [C, C], FP16, isOutput=False)
    projbrow_d = nc.declare_dram_parameter("projbrow", [1, C], F32, isOutput=False)
    biasT_d = nc.declare_dram_parameter(
        "biasT", [H, len(IDENT_KTS) * 128, N], FP8, isOutput=False)
    expbT_d = nc.declare_dram_parameter(
        "expbT", [H, len(EXPB_KTS) * 128, N], BF16, isOutput=False)
    bsum32_d = nc.declare_dram_parameter("bsum32", [128, 32], FP16, isOutput=False)
    identf8_d = nc.declare_dram_parameter("identf8", [128, 128], FP8, isOutput=False)
    rrd_d = nc.declare_dram_parameter("rrd", [4 * CB, N], FP16, isOutput=True)
    out_d = nc.declare_dram_parameter("out", [N, C], F32, isOutput=True)

    with tile.TileContext(nc) as tc:
        with (
            tc.tile_pool(name="persist", bufs=1) as persist,
            tc.tile_pool(name="sqp", bufs=2) as sqp,
            tc.tile_pool(name="psbp", bufs=2) as psbp,
            tc.tile_pool(name="ebias", bufs=2) as ebias,
            tc.tile_pool(name="expt", bufs=2) as expt_pool,
            tc.tile_pool(name="small", bufs=1) as small,
            tc.tile_pool(name="osbp", bufs=2) as osbp,
            tc.tile_pool(name="ps_big", bufs=2, space="PSUM") as ps_big,
            tc.tile_pool(name="ps_av", bufs=4, space="PSUM") as ps_av,
        ):
            # ---------------- load constants / weights ----------------
            # big-chunk DMAs (>=1KB contiguous per partition line) split
            # across the two hardware queues: sync gets x cols 0:512 +
            # weights, scalar gets x cols 512:1024 (+ biases later)
            xT = persist.tile([128, CB, N], FP16, tag="xT")
            nc.sync.dma_start(
                out=xT[:, :, 0:512],
                in_=xT_d.rearrange("(cb p) n -> p cb n", p=128)[:, :, 0:512])
            nc.scalar.dma_start(
                out=xT[:, :, 512:1024],
                in_=xT_d.rearrange("(cb p) n -> p cb n", p=128)[:, :, 512:1024])
            wqkT = persist.tile([128, CB, 2 * C], FP16, tag="wqkT")
            nc.sync.dma_start(
                out=wqkT, in_=wqkT_d.rearrange("(cb p) f -> p cb f", p=128))
            wvT = persist.tile([128, CB, C], FP16, tag="wvT")
            nc.sync.dma_start(
                out=wvT, in_=wvT_d.rearrange("(cb p) f -> p cb f", p=128))
            qkb = persist.tile([128, 2 * CB], F32, tag="qkb")
            nc.sync.dma_start(
                out=qkb, in_=qkb_d.rearrange("(fb p) one -> p (fb one)", p=128))
            sclq = persist.tile([128, CB], F32, tag="sclq")
            nc.sync.dma_start(out=sclq, in_=sclq_d[:])
            bsum32 = persist.tile([128, 32], FP16, tag="bsum32")
            nc.sync.dma_start(out=bsum32, in_=bsum32_d[:])
            identf8 = persist.tile([128, 128], FP8, tag="identf8")
            nc.sync.dma_start(out=identf8, in_=identf8_d[:])
            vb_bc2 = persist.tile([128, 2 * C], F32, tag="vb_bc2")
            nc.sync.dma_start(
                out=vb_bc2, in_=vbrow_d[:].to_broadcast((128, 2 * C)))
            qev = persist.tile([128, H], FP16, tag="qev")
            nc.sync.dma_start(out=qev, in_=qev_d[:])

            for rep in range(reps):
                qkT = persist.tile([128, 2 * CB, N], FP16, tag="qkT")
                qsTd = persist.tile([128, H, N], FP16, tag="qsTd")
                knTd = persist.tile([128, H, N], FP16, tag="knTd")
                v_sb = persist.tile([128, NT, H, HD + 1], BF16, tag="v_sb")
                qekn = persist.tile([128, H, NT], F32, tag="qekn")
                outhT = persist.tile([128, CB, N], FP16, tag="outhT")

                def qkv(hp):
                    # q (fb=hp) and k (fb=CB+hp) projections.  Both q blocks
                    # accumulate into ONE [128,1024] psum tile (8 matmuls)
                    # drained by ONE wide eviction, so the eviction round
                    # trip is fully hidden even with 2 psum bufs.
                    for half in range(2):
                        fb = half * CB + hp
                        ps = ps_big.tile(
                            [128, 1024], F32, tag="ps_big",
                            name=f"psqkv{hp}{half}")
                        for qb in range(QB):
                            for cb in range(CB):
                                nc.tensor.matmul(
                                    ps[:, qb * 512:(qb + 1) * 512],
                                    wqkT[:, cb, fb * 128:(fb + 1) * 128],
                                    xT[:, cb, qb * 512:(qb + 1) * 512],
                                    start=(cb == 0), stop=(cb == CB - 1),
                                )
                        nc.vector.tensor_scalar(
                            out=qkT[:, fb, :],
                            in0=ps, scalar1=qkb[:, fb:fb + 1],
                            scalar2=None, op0=ALU.add)

                def vproj():
                    nc.vector.memset(v_sb[:, :, :, HD:HD + 1], 1.0)
                    for tb in range(0, NT, 2):
                        ps = ps_big.tile([128, 1024], F32, tag="ps_big",
                                         name=f"psv{tb}")
                        for t in range(2):
                            for cb in range(CB):
                                nc.tensor.matmul(
                                    ps[:, t * 512:(t + 1) * 512],
                                    xT[:, cb, (tb + t) * 128:(tb + t + 1) * 128],
                                    wvT[:, cb, :],
                                    start=(cb == 0), stop=(cb == CB - 1),
                                )
                        nc.vector.tensor_add(
                            v_sb[:, tb:tb + 2, :, 0:HD],
                            ps.rearrange("p (t h d) -> p t h d", t=2, h=H),
                            vb_bc2.rearrange("p (t h d) -> p t h d", t=2, h=H),
                        )

                def norms_mm(hp):
                    # squared sums for q and k halves land in ONE psum tile
                    # (q heads at partitions 0-1, k heads at 32-33, rest
                    # zeroed by the 32-wide zero-padded stationary), then a
                    # single wide Sqrt covers both.
                    sqs = []
                    for half in range(2):
                        fb = half * CB + hp
                        sq = sqp.tile([128, N], FP16, tag="sq",
                                      name=f"sq{hp}{half}")
                        nc.vector.tensor_mul(sq, qkT[:, fb, :], qkT[:, fb, :])
                        sqs.append(sq)
                    nps = ps_big.tile([128, 1024], F32, tag="ps_big",
                                      name=f"nps{hp}")
                    for half in range(2):
                        for qb in range(QB):
                            nc.tensor.matmul(
                                nps[32 * half:32 * half + 32,
                                    qb * 512:(qb + 1) * 512],
                                bsum32, sqs[half][:, qb * 512:(qb + 1) * 512],
                                start=True, stop=True)
                    srt = sqp.tile([64, N], F32, tag="srt", bufs=4,
                                   name=f"srt{hp}")
                    nc.scalar.activation(
                        out=srt, in_=nps[0:64, :],
                        func=AF.Sqrt, bias=0.0, scale=1.0)
                    return srt

                def norms_chain(hp, srt):
                    # custom DVE ops misread at partition offsets: run ONE
                    # base-0 reciprocal over rows 0-33 (rows 2-31 are zeros
                    # whose inf recips get clamped and never used)
                    rr = sqp.tile([34, N], F32, tag="rr", bufs=1,
                                  name=f"rr{hp}")
                    nc.vector.reciprocal_approx_fast(rr, srt[0:34, :])
                    rr16 = sqp.tile([34, N], FP16, tag="rr16",
                                    name=f"rr16{hp}")
                    nc.vector.tensor_scalar(
                        out=rr16, in0=rr, scalar1=1e12, scalar2=None,
                        op0=ALU.min)
                    nc.gpsimd.dma_start(
                        out=rrd_d[hp * 4:hp * 4 + 2, :], in_=rr16[0:2, :])
                    nc.gpsimd.dma_start(
                        out=rrd_d[hp * 4 + 2:hp * 4 + 4, :],
                        in_=rr16[32:34, :])

                def scaleqk2(hp):
                    # broadcast the norm recips back from DRAM scratch with
                    # stride-0 reads (rides the same gpsimd hwdge queue as
                    # the writes in norms2, so ordering is FIFO-safe), then
                    # scale both heads of the pair in one [128,1024] op.
                    kn_packed = None
                    for half, dst in ((0, qsTd), (1, knTd)):
                        fb = half * CB + hp
                        r0 = hp * 4 + half * 2
                        psb = psbp.tile([128, N], FP16, tag="psb",
                                        name=f"psb{hp}{half}")
                        nc.gpsimd.dma_start(
                            out=psb[0:64, :],
                            in_=rrd_d[r0:r0 + 1, :].to_broadcast((64, N)))
                        nc.gpsimd.dma_start(
                            out=psb[64:128, :],
                            in_=rrd_d[r0 + 1:r0 + 2, :].to_broadcast((64, N)))
                        packed = psbp.tile([128, N], FP16, tag="packed",
                                           name=f"packed{hp}{half}")
                        if half == 0:
                            nc.vector.scalar_tensor_tensor(
                                out=packed, in0=qkT[:, fb, :],
                                scalar=sclq[:, hp:hp + 1], in1=psb,
                                op0=ALU.mult, op1=ALU.mult)
                        else:
                            nc.vector.tensor_mul(packed, qkT[:, fb, :], psb)
                            kn_packed = packed
                        # duplicate each head's 64 rows into both row halves
                        # so score matmuls can row-pack the two q blocks
                        # (spread across both hw queues)
                        for s in range(2):
                            h = 2 * hp + s
                            nc.sync.dma_start(
                                out=dst[:, h, :][0:64],
                                in_=packed[s * 64:(s + 1) * 64, :])
                            nc.scalar.dma_start(
                                out=dst[:, h, :][64:128],
                                in_=packed[s * 64:(s + 1) * 64, :])
                    return kn_packed

                def qekn_calc(hp, kn_packed):
                    # reads the PACKED kn tile directly (head s on row half
                    # s), so qekn doesn't wait for the knTd extract DMAs
                    for s in range(2):
                        h = 2 * hp + s
                        ro = s * 64
                        psq = ps_big.tile([128, 1024], F32, tag="ps_big",
                                          name=f"psq{h}")
                        for kt in range(NT):
                            nc.tensor.matmul(
                                psq[:, 2 * kt:2 * kt + 1],
                                kn_packed[:, kt * 128:(kt + 1) * 128][ro:ro + 64],
                                qev[ro:ro + 64, h:h + 1],
                                start=True, stop=True)
                        nc.vector.tensor_copy(
                            qekn[:, h, :],
                            psq[:, 0:2 * NT].rearrange(
                                "p (a two) -> p a two", two=2)[:, :, 0])

                bias_tiles = {}

                def prefetch_bias(h):
                    # bias DMAs ride the scalar (ACT) hwdge queue so the big
                    # fp8 streams never block weight loads on the sync queue
                    bt = ebias.tile([128, len(IDENT_KTS), N], FP8, tag="bias",
                                    name=f"bias{h}")
                    nc.scalar.dma_start(
                        out=bt,
                        in_=biasT_d[h].rearrange("(kt p) q -> p kt q", p=128))
                    ebt = ebias.tile([128, len(EXPB_KTS), N], BF16,
                                     tag="expb", name=f"expb{h}")
                    nc.scalar.dma_start(
                        out=ebt,
                        in_=expbT_d[h].rearrange("(kt p) q -> p kt q", p=128))
                    bias_tiles[h] = (bt, ebt)

                pav_tiles = {}

                def group_main(h):
                    # attention for one head, both q blocks at once
                    bias_t, expb_t = bias_tiles.pop(h)
                    eT = expt_pool.tile([128, NT, N], BF16, tag="eT",
                                        name=f"eT{h}")
                    pavs = [ps_av.tile([HD + 1, 512], F32, tag="ps_av",
                                       name=f"pav{h}{i}") for i in range(2)]
                    pav_tiles[h] = pavs

                    def scores(kt):
                        ident = kt in IDENT_KTS
                        ps = ps_big.tile([128, 1024], F32, tag="ps_big",
                                         name=f"pssc{h}{kt}")
                        if ident:
                            j = IDENT_KTS.index(kt)
                            for qb in range(QB):
                                nc.tensor.matmul(
                                    ps[:, qb * 512:(qb + 1) * 512], identf8,
                                    bias_t[:, j, qb * 512:(qb + 1) * 512],
                                    start=True, stop=False)
                        # row-packed pair: qb0 on rows 0-63, qb1 on 64-127
                        for qb in range(QB):
                            ro = qb * 64
                            nc.tensor.matmul(
                                ps[:, qb * 512:(qb + 1) * 512],
                                knTd[:, h, kt * 128:(kt + 1) * 128][ro:ro + 64],
                                qsTd[:, h,
                                     qb * 512:(qb + 1) * 512][ro:ro + 64],
                                start=not ident, stop=True,
                            )
                        nc.scalar.activation(
                            out=eT[:, kt, :], in_=ps, func=AF.Exp,
                            bias=qekn[:, h, kt:kt + 1], scale=LN2)
                        if kt in DVE_KTS:
                            nc.vector.tensor_mul(
                                eT[:, kt, :], eT[:, kt, :],
                                expb_t[:, EXPB_IDX[kt], :])
                        elif kt in GP_KTS:
                            nc.gpsimd.tensor_mul(
                                eT[:, kt, :], eT[:, kt, :],
                                expb_t[:, EXPB_IDX[kt], :])

                    def av(kt):
                        for qb in range(QB):
                            nc.tensor.matmul(
                                pavs[qb],
                                v_sb[:, kt, h, :],
                                eT[:, kt, qb * 512:(qb + 1) * 512],
                                start=(kt == 0), stop=(kt == NT - 1),
                            )

                    # AV lags scores by 3 kt so earlier divisions (which hold
                    # the pav slots, 4 bufs deep) have time to finish
                    for kt in range(NT):
                        scores(kt)
                        if kt >= 3:
                            av(kt - 3)
                    for kt in range(NT - 3, NT):
                        av(kt)

                def group_div(h):
                    # division for one head, both q blocks batched:
                    # one reciprocal + one partition broadcast
                    hp, sub = h // 2, h % 2
                    po = sub * 64
                    pav0, pav1 = pav_tiles[h]
                    den = small.tile([1, 1024], F32, tag="dens",
                                     name=f"den{h}")
                    nc.vector.tensor_copy(den[:, 0:512], pav0[HD:HD + 1, :])
                    nc.vector.tensor_copy(den[:, 512:1024], pav1[HD:HD + 1, :])
                    rrec = small.tile([1, 1024], F32, tag="rrec",
                                      name=f"rrec{h}")
                    nc.vector.reciprocal_approx_fast(rrec, den)
                    rrb = small.tile([HD, 1024], F32, tag="rrb",
                                     name=f"rrb{h}")
                    nc.gpsimd.partition_broadcast(rrb, rrec)
                    for qb, pav in ((0, pav0), (1, pav1)):
                        nc.vector.scalar_tensor_tensor(
                            out=outhT[:, hp,
                                      qb * 512:(qb + 1) * 512][po:po + 64],
                            in0=pav[0:HD, :], scalar=1.0,
                            in1=rrb[:, qb * 512:(qb + 1) * 512],
                            op0=ALU.mult, op1=ALU.mult)

                def group_div_qb(h, qb):
                    # unbatched variant for the final head so proj can start
                    # after the first q block's division
                    hp, sub = h // 2, h % 2
                    po = sub * 64
                    pav = pav_tiles[h][qb]
                    den = small.tile([1, 512], F32, tag="dens1",
                                     name=f"den1{h}{qb}")
                    nc.vector.tensor_copy(den, pav[HD:HD + 1, :])
                    rrec = small.tile([1, 512], F32, tag="rrec1",
                                      name=f"rrec1{h}{qb}")
                    nc.vector.reciprocal_approx_fast(rrec, den)
                    rrb = small.tile([HD, 512], F32, tag="rrb1",
                                     name=f"rrb1{h}{qb}")
                    nc.gpsimd.partition_broadcast(rrb, rrec)
                    nc.vector.scalar_tensor_tensor(
                        out=outhT[:, hp, qb * 512:(qb + 1) * 512][po:po + 64],
                        in0=pav[0:HD, :], scalar=1.0, in1=rrb,
                        op0=ALU.mult, op1=ALU.mult)

                def proj(tb):
                    ps = ps_big.tile([128, 1024], F32, tag="ps_big",
                                     name=f"pso{tb}")
                    for fb in range(CB):
                        nc.tensor.matmul(
                            ps[:, 0:512],
                            outhT[:, fb, tb * 128:(tb + 1) * 128],
                            projwT[:, fb, :],
                            start=(fb == 0), stop=(fb == CB - 1),
                        )
                    osb = osbp.tile([128, C], F32, tag="osb", name=f"osb{tb}")
                    nc.vector.tensor_add(osb, ps[:, 0:512], projb_bc)
                    nc.sync.dma_start(
                        out=out_d[tb * 128:(tb + 1) * 128, :], in_=osb)

                # prologue: all four head-pairs' QKV + norm chains run before
                # any attention so the ACT queue is [sqrt x4, exp x64] with
                # exactly two table loads.  All QKV psum evictions and sq
                # squares are emitted BEFORE any norm-chain DVE op so the
                # chains never head-of-line-block the PE's psum recycling.
                # bias/proj weight DMAs are emitted late so the scheduler
                # can't run them ahead of the critical x/w loads.
                qkv(0)
                qkv(1)
                srt0 = norms_mm(0)
                qkv(2)
                srt1 = norms_mm(1)
                qkv(3)
                srt2 = norms_mm(2)
                srt3 = norms_mm(3)
                norms_chain(0, srt0)
                qekn_calc(0, scaleqk2(0))
                norms_chain(1, srt1)
                qekn_calc(1, scaleqk2(1))
                prefetch_bias(0)
                prefetch_bias(1)
                norms_chain(2, srt2)
                qekn_calc(2, scaleqk2(2))
                norms_chain(3, srt3)
                qekn_calc(3, scaleqk2(3))
                vproj()
                projwT = persist.tile([128, CB, C], FP16, tag="projwT")
                nc.scalar.dma_start(
                    out=projwT,
                    in_=projwT_d.rearrange("(cb p) f -> p cb f", p=128))
                projb_bc = persist.tile([128, C], F32, tag="projb_bc")
                nc.scalar.dma_start(
                    out=projb_bc, in_=projbrow_d[:].to_broadcast((128, C)))
                for h in range(H):
                    if h + 2 < H:
                        prefetch_bias(h + 2)
                    # division for the previous head is emitted BEFORE this
                    # head's group so its gpsimd broadcast isn't queued
                    # behind this head's gpsimd eT multiply
                    if h >= 1:
                        group_div(h - 1)
                    group_main(h)
                group_div_qb(H - 1, 0)
                for tb in range(NT // 2):
                    proj(tb)
                group_div_qb(H - 1, 1)
                for tb in range(NT // 2, NT):
                    proj(tb)

    nc.compile()
    return nc


def _host_prep(inputs):
    """Host-side layout/scalar prep. Returns per-core input maps."""
    x = np.asarray(inputs["x"], dtype=np.float32)
    qkv_w = np.asarray(inputs["qkv_w"], dtype=np.float32)
    qkv_b = np.asarray(inputs["qkv_b"], dtype=np.float32)
    proj_w = np.asarray(inputs["proj_w"], dtype=np.float32)
    proj_b = np.asarray(inputs["proj_b"], dtype=np.float32)
    temp = np.asarray(inputs["temperature"], dtype=np.float32).reshape(H)
    qe = np.asarray(inputs["query_embedding"], dtype=np.float32).reshape(H, HD)
    tab = np.asarray(inputs["relative_coords_table"], dtype=np.float32)
    idx = np.asarray(inputs["relative_pos_index"])
    f1w = np.asarray(inputs["cpb_fc1_w"], dtype=np.float32)
    f1b = np.asarray(inputs["cpb_fc1_b"], dtype=np.float32)
    f2w = np.asarray(inputs["cpb_fc2_w"], dtype=np.float32)
    f2b = np.asarray(inputs["cpb_fc2_b"], dtype=np.float32)
    sls = np.asarray(inputs["seq_length_scale"], dtype=np.float32)

    scale = (np.logaddexp(0.0, temp) * sls[0]).astype(np.float32)

    hidden = np.maximum(tab @ f1w.T + f1b, 0.0)
    bias_tab = ((hidden @ f2w.T + f2b) * LOG2E).astype(np.float32)  # (T, H)
    bias = bias_tab[idx]                                            # (q, k, H)
    biasT = np.ascontiguousarray(np.transpose(bias, (2, 1, 0)))     # (H, k, q)
    b4 = biasT.reshape(H, NT, 128, N)
    expbT = np.exp(b4[:, EXPB_KTS].astype(np.float32) * LN2).astype(
        NB_BF16).reshape(H, len(EXPB_KTS) * 128, N)
    biasT = np.ascontiguousarray(b4[:, IDENT_KTS]).astype(NB_FP8).reshape(
        H, len(IDENT_KTS) * 128, N)

    wqkT = np.ascontiguousarray(qkv_w[:2 * C].T).astype(NB_FP16)   # (cin, 1024)
    wvT = np.ascontiguousarray(qkv_w[2 * C:].T).astype(NB_FP16)    # (cin, 512)
    projwT = np.ascontiguousarray(proj_w.T).astype(NB_FP16)        # (cin, 512)
    qkb = qkv_b[:2 * C].reshape(2 * C, 1).copy()
    vbrow = np.tile(qkv_b[2 * C:].reshape(1, C), (1, 2)).copy()
    projbrow = proj_b.reshape(1, C).copy()
    # qev[d, h] = qe[h, d]*scale[h]; knTd[0:64].T @ qev[:, h] gives the
    # per-key k_hat . qe*scale offset.
    qev64 = np.ascontiguousarray((qe * scale[:, None]).T).astype(NB_FP16)
    qev = np.vstack([qev64, qev64])  # duplicated so row half s=1 can read it
    # sclq[p, hp] = scale[2*hp + (p>=64)] * log2e, applied per feature row
    # during the packed q-scale STT
    sclq = np.zeros((128, CB), dtype=np.float32)
    for hp in range(CB):
        sclq[0:64, hp] = scale[2 * hp] * LOG2E
        sclq[64:128, hp] = scale[2 * hp + 1] * LOG2E

    # cols 0/1 sum the two head-halves; cols 2-31 repeat them so the psum
    # pad rows hold real positive norms (no NaN from recip of zeros)
    bsum32 = np.zeros((128, 32), dtype=NB_FP16)
    bsum32[:64, 0::2] = 1.0
    bsum32[64:, 1::2] = 1.0
    identf8 = np.eye(128, dtype=NB_FP8)

    shared = dict(
        wqkT=wqkT, wvT=wvT, qkb=qkb, vbrow=vbrow, qev=qev,
        sclq=sclq, projwT=projwT, projbrow=projbrow, biasT=biasT,
        expbT=expbT, bsum32=bsum32, identf8=identf8,
    )
    in_maps = []
    for b in range(B):
        m = dict(shared)
        m["xT"] = np.ascontiguousarray(x[b].T).astype(NB_FP16)
        in_maps.append(m)
    return in_maps


def get_nc(reps=1):
    key = ("nc", reps)
    if key not in _CACHE:
        _CACHE[key] = _build(reps)
    return _CACHE[key]


def kernel(**inputs) -> np.ndarray:
    nc = get_nc()
    in_maps = _host_prep(inputs)
    res = run_bass_kernel_spmd(nc, in_maps, core_ids=list(range(B)))
    out = np.stack([res.results[b]["out"] for b in range(B)], axis=0)
    return out.astype(np.float32)
